# revision 1
# baseline (speedup 1.0000x reference)
"""GQA kernel for Trainium2, 8 NeuronCores (DP over batch x TP over heads).

Problem (hardcoded): B=4, S=1024, EMBED=2048, HEADS=32, GROUPS=8,
GROUP_HEADS=4, HEAD_DIM=64.

Sharding: core c handles batch b = c//2 and TP half m = c%2
(16 heads = 4 groups per core). All tensors are pre-transposed on the
host so the device only ever sees contract-dim-major operands:

  qT/kT/vT  [2048, 1024]   (embed-major tokens for one batch)
  wqT       [2048, 1024]   (Wq rows m*1024:(m+1)*1024, transposed, pre-scaled 1/8)
  wkT/wvT   [2048, 256]    (Wk/Wv rows m*256:(m+1)*256, transposed)
  wfcT      [1024, 2048]   (Wfc columns m*1024:(m+1)*1024, transposed)
  y         [1024, 2048]   partial output; host computes y[2b] + y[2b+1] + bfc.

Device pipeline per core (all matmuls fp32r):
  1. khT [256,1024] and vh [1024,256] projections; khT duplicated into
     per-group [128,1024] tiles (both 64-partition halves hold the same
     group) so score matmuls can run at either array quadrant; vh gets
     a ones column (AV matmul then emits softmax denominators for free).
  2. qhT [1024,1024] projection (head-dim-major).
  3. Per head: scores = khT_g.T @ qhT_h -> exp (ACT, no max subtraction:
     |score| <= ~6 by construction) -> AV accumulation (vh as stationary)
     -> normalize by denominator broadcast (DMA partition-replicate +
     DVE reciprocal/mul) into OT [1024,1024].
  4. y = OT.T @ wfcT accumulated over 8 i-chunks.
"""

import numpy as np

import concourse.bass as bass
import concourse.tile as tile
from concourse import bacc, mybir
from concourse.bass_utils import run_bass_kernel_spmd

F32 = mybir.dt.float32
F32R = mybir.dt.float32r
AF = mybir.ActivationFunctionType

B, S, E = 4, 1024, 2048
HEADS_L = 16          # heads per core
GROUPS_L = 4          # groups per core
D = 64                # head dim
P = 128
NE = E // P           # 16 e-chunks
NT = S // P           # 8 token chunks
HO = HEADS_L * D      # 1024 local head-dims
GO = GROUPS_L * D     # 256 local group-dims

_CACHE = {}


def _build():
    nc = bacc.Bacc("TRN2")
    qT = nc.declare_dram_parameter("qT", [E, S], F32R, isOutput=False)
    kT = nc.declare_dram_parameter("kT", [E, S], F32R, isOutput=False)
    vT = nc.declare_dram_parameter("vT", [E, S], F32R, isOutput=False)
    wqT = nc.declare_dram_parameter("wqT", [E, HO], F32R, isOutput=False)
    wkT = nc.declare_dram_parameter("wkT", [E, GO], F32R, isOutput=False)
    wvT = nc.declare_dram_parameter("wvT", [E, GO], F32R, isOutput=False)
    wfcT = nc.declare_dram_parameter("wfcT", [HO, E], F32R, isOutput=False)
    y = nc.declare_dram_parameter("y", [S, E], F32, isOutput=True)
    dbg = {}
    if _CACHE.get("debug"):
        for nm, shp in [("dqh", [P, S]), ("dkh", [P, S]), ("dvh", [P, GROUPS_L * (D + 1)]),
                        ("dexp", [P, S]), ("drecip", [P, S]), ("dot", [P, S])]:
            dbg[nm] = nc.declare_dram_parameter(nm, shp, F32, isOutput=True)

    with tile.TileContext(nc) as tc:
        _body(nc, tc, qT, kT, vT, wqT, wkT, wvT, wfcT, y, dbg)
    nc.finalize()
    return nc


def _body(nc, tc, qT, kT, vT, wqT, wkT, wvT, wfcT, y, dbg=None):
    dbg = dbg or {}
    from contextlib import ExitStack
    with ExitStack() as ctx:
        # persistent pools (whole kernel lifetime)
        p_kh = ctx.enter_context(tc.tile_pool(name="kh", bufs=GROUPS_L))
        p_vh = ctx.enter_context(tc.tile_pool(name="vh", bufs=NT))
        p_qh = ctx.enter_context(tc.tile_pool(name="qh", bufs=NT))
        p_ot = ctx.enter_context(tc.tile_pool(name="ot", bufs=NT))
        p_wfc = ctx.enter_context(tc.tile_pool(name="wfc", bufs=8))
        p_y = ctx.enter_context(tc.tile_pool(name="y", bufs=2))
        ps = ctx.enter_context(tc.tile_pool(name="ps", bufs=8, space="PSUM"))

        kh_dup = [p_kh.tile([P, S], F32R, tag="kh", name=f"khdup_{g}") for g in range(GROUPS_L)]
        vh_aug = [p_vh.tile([P, GROUPS_L, D + 1], F32R, tag="vh", name=f"vhaug_{t}")
                  for t in range(NT)]
        qh_t = [p_qh.tile([P, S], F32R, tag="qh", name=f"qh_{t}") for t in range(NT)]
        ot_t = [p_ot.tile([P, S], F32R, tag="ot", name=f"ot_{t}") for t in range(NT)]

        with tc.tile_pool(name="wk", bufs=NE) as p_wk, \
             tc.tile_pool(name="wv", bufs=NE) as p_wv, \
             tc.tile_pool(name="kv", bufs=4) as p_kv, \
             tc.tile_pool(name="wq", bufs=4) as p_wq:
            # ---- stage 1: K/V projections ---------------------------
            wk_t = []
            wv_t = []
            kh_ps = [[ps.tile([P, 512], F32, tag="ps", name=f"khps_{a}_{b}") for b in range(2)]
                     for a in range(2)]
            for e in range(NE):
                wkt = p_wk.tile([P, GO], F32R, tag="wk")
                nc.sync.dma_start(out=wkt, in_=wkT[e * P:(e + 1) * P, :])
                wk_t.append(wkt)
                kte = p_kv.tile([P, S], F32R, tag="kv", name=f"kte_{e}")
                nc.sync.dma_start(out=kte, in_=kT[e * P:(e + 1) * P, :])
                wvt = p_wv.tile([P, GO], F32R, tag="wv")
                nc.sync.dma_start(out=wvt, in_=wvT[e * P:(e + 1) * P, :])
                wv_t.append(wvt)
                for o2 in range(2):
                    for t2 in range(2):
                        nc.tensor.matmul(
                            kh_ps[o2][t2][:, :],
                            wk_t[e][:, o2 * P:(o2 + 1) * P],
                            kte[:, t2 * 512:(t2 + 1) * 512],
                            start=(e == 0), stop=(e == NE - 1),
                        )
            for o2 in range(2):
                for t2 in range(2):
                    sl = slice(t2 * 512, (t2 + 1) * 512)
                    nc.vector.tensor_copy(kh_dup[2 * o2][0:D, sl],
                                          kh_ps[o2][t2][0:D, :])
                    nc.vector.tensor_copy(kh_dup[2 * o2 + 1][D:P, sl],
                                          kh_ps[o2][t2][D:P, :])
            for g in range(GROUPS_L):
                if g % 2 == 0:
                    nc.gpsimd.dma_start(out=kh_dup[g][D:P, :], in_=kh_dup[g][0:D, :])
                else:
                    nc.gpsimd.dma_start(out=kh_dup[g][0:D, :], in_=kh_dup[g][D:P, :])

            vh_ps = [ps.tile([P, GO], F32, tag="ps", name=f"vhps_{t}") for t in range(NT)]
            for e in range(NE):
                vte = p_kv.tile([P, S], F32R, tag="kv", name=f"vte_{e}")
                nc.sync.dma_start(out=vte, in_=vT[e * P:(e + 1) * P, :])
                for t in range(NT):
                    nc.tensor.matmul(
                        vh_ps[t][:, :],
                        vte[:, t * P:(t + 1) * P],
                        wv_t[e][:, :],
                        start=(e == 0), stop=(e == NE - 1),
                    )
            for t in range(NT):
                for g in range(GROUPS_L):
                    nc.vector.tensor_copy(vh_aug[t][:, g, 0:D],
                                          vh_ps[t][:, g * D:(g + 1) * D])
                ones = nc.const_aps.tensor(1.0, (P, 1), F32)
                for g in range(GROUPS_L):
                    nc.vector.tensor_copy(vh_aug[t][:, g, D:D + 1], ones)

            # ---- stage 2: Q projection ------------------------------
            for rnd in range(2):
                wq_r = []
                for e in range(NE):
                    wqe = p_wq.tile([P, 512], F32R, tag="wq", name=f"wq_{rnd}_{e}")
                    nc.sync.dma_start(
                        out=wqe,
                        in_=wqT[e * P:(e + 1) * P, rnd * 512:(rnd + 1) * 512])
                    wq_r.append(wqe)
                qps = [[ps.tile([P, 512], F32, tag="ps", name=f"qps_{a}_{b}") for b in range(2)]
                       for a in range(4)]
                for e in range(NE):
                    qte = p_kv.tile([P, S], F32R, tag="kv", name=f"qte_{rnd}_{e}")
                    nc.sync.dma_start(out=qte, in_=qT[e * P:(e + 1) * P, :])
                    for o in range(4):
                        for t2 in range(2):
                            nc.tensor.matmul(
                                qps[o][t2][:, :],
                                wq_r[e][:, o * P:(o + 1) * P],
                                qte[:, t2 * 512:(t2 + 1) * 512],
                                start=(e == 0), stop=(e == NE - 1),
                            )
                for o in range(4):
                    for t2 in range(2):
                        nc.scalar.activation(
                            qh_t[rnd * 4 + o][:, t2 * 512:(t2 + 1) * 512],
                            qps[o][t2][:, :], AF.Copy)

        if dbg:
            nc.sync.dma_start(out=dbg["dqh"][:, :], in_=qh_t[0][:, :].bitcast(F32))
            nc.sync.dma_start(out=dbg["dkh"][:, :], in_=kh_dup[0][:, :].bitcast(F32))
            nc.sync.dma_start(out=dbg["dvh"][:, :], in_=vh_aug[0].rearrange("p g d -> p (g d)").bitcast(F32))

        # ---- stage 3: attention per head ----------------------------
        with tc.tile_pool(name="exp", bufs=10) as p_exp, \
             tc.tile_pool(name="sm", bufs=3) as p_sm:
            for h in range(HEADS_L):
                g = h // 4
                qtile = qh_t[h // 2]
                qb = (h % 2) * D  # partition base inside qh tile

                exp_t = [p_exp.tile([P, S], F32R, tag="exp", name=f"exp_{h}_{kc}") for kc in range(NT)]
                for kc in range(NT):
                    for q2 in range(2):
                        sps = ps.tile([P, 512], F32, tag="ps", name=f"sps_{h}_{kc}_{q2}")
                        nc.tensor.matmul(
                            sps[:, :],
                            kh_dup[g][qb:qb + D, kc * P:(kc + 1) * P],
                            qtile[qb:qb + D, q2 * 512:(q2 + 1) * 512],
                            start=True, stop=True,
                        )
                        nc.scalar.activation(
                            exp_t[kc][:, q2 * 512:(q2 + 1) * 512], sps[:, :],
                            AF.Exp)

                den = p_sm.tile([P, S], F32, tag="den", name=f"den_{h}")
                av_ps = []
                for q2 in range(2):
                    ops = ps.tile([P, 512], F32, tag="ps", name=f"avps_{h}_{q2}")
                    for kc in range(NT):
                        nc.tensor.matmul(
                            ops[0:D + 1, :],
                            vh_aug[kc][:, g, :],
                            exp_t[kc][:, q2 * 512:(q2 + 1) * 512],
                            start=(kc == 0), stop=(kc == NT - 1),
                        )
                    nc.vector.tensor_copy(den[D:D + 1, q2 * 512:(q2 + 1) * 512],
                                          ops[D:D + 1, :])
                    av_ps.append(ops)
                recip = p_sm.tile([P, S], F32, tag="recip", name=f"recip_{h}")
                nc.gpsimd.dma_start(out=den[0:1, :], in_=den[D:D + 1, :])
                nc.gpsimd.partition_broadcast(recip[0:D, :], den[0:1, :])
                nc.vector.reciprocal(recip[0:D, :], recip[0:D, :])
                if dbg and h == 0:
                    nc.sync.dma_start(out=dbg["dexp"][:, :], in_=exp_t[0][:, :].bitcast(F32))
                    nc.sync.dma_start(out=dbg["drecip"][:, :], in_=recip[:, :])
                if h % 2 == 0:
                    for q2 in range(2):
                        sl = slice(q2 * 512, (q2 + 1) * 512)
                        nc.vector.tensor_mul(ot_t[h // 2][0:D, sl],
                                             av_ps[q2][0:D, :], recip[0:D, sl])
                else:
                    tmp = p_sm.tile([P, S], F32R, tag="tmp", name=f"tmp_{h}")
                    for q2 in range(2):
                        sl = slice(q2 * 512, (q2 + 1) * 512)
                        nc.vector.tensor_mul(tmp[0:D, sl],
                                             av_ps[q2][0:D, :], recip[0:D, sl])
                    nc.gpsimd.dma_start(out=ot_t[h // 2][D:P, :], in_=tmp[0:D, :])

            if dbg:
                nc.sync.dma_start(out=dbg["dot"][:, :], in_=ot_t[0][:, :].bitcast(F32))

        # ---- stage 4: output projection (four out-quarter rounds) ---
        for r in range(4):
            wfc_t = []
            for i in range(NT):
                wfct = p_wfc.tile([P, 512], F32R, tag="wfc", name=f"wfc_{r}_{i}")
                nc.sync.dma_start(
                    out=wfct,
                    in_=wfcT[i * P:(i + 1) * P, r * 512:(r + 1) * 512])
                wfc_t.append(wfct)
            for t in range(NT):
                y_sb = p_y.tile([P, 512], F32, tag="y", name=f"ysb_{r}_{t}")
                yps = ps.tile([P, 512], F32, tag="ps", name=f"yps_{r}_{t}")
                for i in range(NT):
                    nc.tensor.matmul(
                        yps[:, :],
                        ot_t[i][:, t * P:(t + 1) * P],
                        wfc_t[i][:, r * 0:512],
                        start=(i == 0), stop=(i == NT - 1),
                    )
                nc.scalar.activation(y_sb[:, :], yps[:, :], AF.Copy)
                nc.sync.dma_start(out=y[t * P:(t + 1) * P, r * 512:(r + 1) * 512],
                                  in_=y_sb)


def _get_nc():
    if "nc" not in _CACHE:
        _CACHE["nc"] = _build()
    return _CACHE["nc"]


def kernel(q, k, v, Wq, Wk, Wv, Wfc, bfc):
    q = np.asarray(q, np.float32)
    k = np.asarray(k, np.float32)
    v = np.asarray(v, np.float32)
    Wq = np.asarray(Wq, np.float32)
    Wk = np.asarray(Wk, np.float32)
    Wv = np.asarray(Wv, np.float32)
    Wfc = np.asarray(Wfc, np.float32)
    bfc = np.asarray(bfc, np.float32)

    nc = _get_nc()
    qTb = [np.ascontiguousarray(q[b].T) for b in range(B)]
    kTb = [np.ascontiguousarray(k[b].T) for b in range(B)]
    vTb = [np.ascontiguousarray(v[b].T) for b in range(B)]
    wqTm = [np.ascontiguousarray((Wq[m * HO:(m + 1) * HO, :] / 8.0).T)
            for m in range(2)]
    wkTm = [np.ascontiguousarray(Wk[m * GO:(m + 1) * GO, :].T) for m in range(2)]
    wvTm = [np.ascontiguousarray(Wv[m * GO:(m + 1) * GO, :].T) for m in range(2)]
    wfcTm = [np.ascontiguousarray(Wfc[:, m * HO:(m + 1) * HO].T)
             for m in range(2)]

    in_maps = []
    for c in range(8):
        b, m = c // 2, c % 2
        in_maps.append({
            "qT": qTb[b], "kT": kTb[b], "vT": vTb[b],
            "wqT": wqTm[m], "wkT": wkTm[m], "wvT": wvTm[m],
            "wfcT": wfcTm[m],
        })
    res = run_bass_kernel_spmd(nc, in_maps, list(range(8)))
    out = np.empty((B, S, E), np.float32)
    for b in range(B):
        out[b] = res.results[2 * b]["y"] + res.results[2 * b + 1]["y"] + bfc
    return out



# revision 15
# speedup vs baseline: 1.3687x; 1.3687x over previous
"""GQA kernel for Trainium2, 8 NeuronCores (DP over batch x TP over heads).

Problem (hardcoded): B=4, S=1024, EMBED=2048, HEADS=32, GROUPS=8,
GROUP_HEADS=4, HEAD_DIM=64.

Core c handles batch b = c//2 and TP half m = c%2 (16 heads = 4 groups).
All matmul operands are bf16 (PSUM accumulation stays fp32); host converts.

Device pipeline (single dense PE stream to keep the HAM clock gate at 8/8):
  [K proj][V proj][Q proj chunk 0]
  [16 attention slots: slot s = head pair p=s%8, token half = s//8.
     scores for the pair are row-tiled (partitions 0:64 / 64:128) so the
     two heads' 64-contract matmuls run concurrently; exp on ACT reads
     1024-wide (two PSUM banks); AV accumulates [dims|ones] so softmax
     denominators fall out of the matmul; normalization via
     reciprocal_approx_fast + gpsimd partition broadcast.
     Fill work keeps the PE busy under the ACT-bound exp stream:
     slots 0-6 run Q-proj chunk p+1, slots 8-15 run the output projection
     for token half A (2 out-chunks per slot).]
  [FC tail: output projection for token half B]
Output is yT [E, S] bf16 (stationary-wfc FC); host transposes and reduces.
"""

import numpy as np
import ml_dtypes

import concourse.bass as bass
import concourse.tile as tile
from concourse import bacc, mybir
from concourse.bass_utils import run_bass_kernel_spmd

F32 = mybir.dt.float32
BF16 = mybir.dt.bfloat16
AF = mybir.ActivationFunctionType

B, S, E = 4, 1024, 2048
HEADS_L = 16          # heads per core
GROUPS_L = 4          # groups per core
D = 64                # head dim
P = 128
NE = E // P           # 16 e-chunks
NT = S // P           # 8 token chunks
HO = HEADS_L * D      # 1024 local head-dims
GO = GROUPS_L * D     # 256 local group-dims
H2 = S // 2           # 512 = token half

_CACHE = {}
_VARIANT = {"approx_recip": True, "bcast_from": 0}


def _build():
    nc = bacc.Bacc("TRN2")
    qT = nc.declare_dram_parameter("qT", [E, S], BF16, isOutput=False)
    kT = nc.declare_dram_parameter("kT", [E, S], BF16, isOutput=False)
    vT = nc.declare_dram_parameter("vT", [E, S], BF16, isOutput=False)
    wqT = nc.declare_dram_parameter("wqT", [E, HO], BF16, isOutput=False)
    wkT = nc.declare_dram_parameter("wkT", [E, GO], BF16, isOutput=False)
    wvT = nc.declare_dram_parameter("wvT", [E, GO], BF16, isOutput=False)
    wfcT = nc.declare_dram_parameter("wfcT", [HO, E], BF16, isOutput=False)
    y = nc.declare_dram_parameter("y", [E, S], BF16, isOutput=True)

    with tile.TileContext(nc) as tc:
        _body(nc, tc, qT, kT, vT, wqT, wkT, wvT, wfcT, y)
    nc.finalize()
    return nc


def _body(nc, tc, qT, kT, vT, wqT, wkT, wvT, wfcT, y):
    from contextlib import ExitStack
    with ExitStack() as ctx:
        # ---- persistent SBUF pools -----------------------------------
        p_kh = ctx.enter_context(tc.tile_pool(name="kh", bufs=GROUPS_L))
        p_vh = ctx.enter_context(tc.tile_pool(name="vh", bufs=NT))
        p_qh = ctx.enter_context(tc.tile_pool(name="qh", bufs=NT))
        p_ot = ctx.enter_context(tc.tile_pool(name="ot", bufs=NT))
        p_wfc = ctx.enter_context(tc.tile_pool(name="wfc", bufs=NT))
        p_rr = ctx.enter_context(tc.tile_pool(name="rr", bufs=2))
        p_r2 = ctx.enter_context(tc.tile_pool(name="r2", bufs=2))
        p_rb = ctx.enter_context(tc.tile_pool(name="rb", bufs=2))
        p_tmp = ctx.enter_context(tc.tile_pool(name="tmp", bufs=2))
        p_ysb = ctx.enter_context(tc.tile_pool(name="ysb", bufs=4))
        p_exp = ctx.enter_context(tc.tile_pool(name="exp", bufs=6))
        # stream pool reused for kte -> vte -> qte (qte lives to slot 6)
        p_in = ctx.enter_context(tc.tile_pool(name="instream", bufs=NE))
        p_wq = ctx.enter_context(tc.tile_pool(name="wq", bufs=2 * NE))

        kh_dup = [p_kh.tile([P, S], BF16, tag="kh", name=f"khdup_{g}")
                  for g in range(GROUPS_L)]
        vh_aug = [p_vh.tile([P, GROUPS_L, D + 1], BF16, tag="vh", name=f"vhaug_{t}")
                  for t in range(NT)]
        qh_t = [p_qh.tile([P, S], BF16, tag="qh", name=f"qh_{t}") for t in range(NT)]
        ot_t = [p_ot.tile([P, S], BF16, tag="ot", name=f"ot_{t}") for t in range(NT)]
        wfc_t = [p_wfc.tile([P, E], BF16, tag="wfc", name=f"wfc_{i}") for i in range(NT)]
        _CACHE["tiles"] = {"kh": kh_dup, "vh": vh_aug, "qh": qh_t, "ot": ot_t}

        ones_col = nc.const_aps.tensor(1.0, (P, 1), BF16)

        # ---- phase 1: K/V projections --------------------------------
        with tc.tile_pool(name="wk", bufs=NE) as p_wk, \
             tc.tile_pool(name="wv", bufs=NE) as p_wv:
            wk_t, wv_t, kte = [], [], []
            for e in range(NE):
                wkt = p_wk.tile([P, GO], BF16, tag="wk")
                nc.sync.dma_start(out=wkt, in_=wkT[e * P:(e + 1) * P, :])
                wk_t.append(wkt)
                kt = p_in.tile([P, S], BF16, tag="in", name=f"kte_{e}")
                nc.sync.dma_start(out=kt, in_=kT[e * P:(e + 1) * P, :])
                kte.append(kt)
                wvt = p_wv.tile([P, GO], BF16, tag="wv")
                nc.sync.dma_start(out=wvt, in_=wvT[e * P:(e + 1) * P, :])
                wv_t.append(wvt)
            # wfc loads early; consumed from slot 8 on
            for i in range(NT):
                nc.sync.dma_start(out=wfc_t[i], in_=wfcT[i * P:(i + 1) * P, :])

            # K projection: khT [256,1024]; o2-outer so drains overlap
            with tc.tile_pool(name="pk", bufs=2, space="PSUM") as ps_k:
                for o2 in range(2):
                    khps = ps_k.tile([P, S], F32, tag="pk", name=f"khps_{o2}")
                    for e in range(NE):
                        for t2 in range(2):
                            nc.tensor.matmul(
                                khps[:, t2 * H2:(t2 + 1) * H2],
                                wk_t[e][:, o2 * P:(o2 + 1) * P],
                                kte[e][:, t2 * H2:(t2 + 1) * H2],
                                start=(e == 0), stop=(e == NE - 1),
                            )
                    nc.scalar.activation(kh_dup[2 * o2][0:D, :],
                                         khps[0:D, :], AF.Copy)
                    nc.scalar.activation(kh_dup[2 * o2 + 1][D:P, :],
                                         khps[D:P, :], AF.Copy)
                for g in range(GROUPS_L):
                    if g % 2 == 0:
                        nc.gpsimd.dma_start(out=kh_dup[g][D:P, :],
                                            in_=kh_dup[g][0:D, :])
                    else:
                        nc.gpsimd.dma_start(out=kh_dup[g][0:D, :],
                                            in_=kh_dup[g][D:P, :])

                # V projection: vh [tokens, dims]; two token-chunks per bank
                vte = []
                for e in range(NE):
                    vt = p_in.tile([P, S], BF16, tag="in", name=f"vte_{e}")
                    nc.sync.dma_start(out=vt, in_=vT[e * P:(e + 1) * P, :])
                    vte.append(vt)
                for t in range(NT):
                    vps = ps_k.tile([P, GO], F32, tag="pv", bufs=4,
                                    name=f"vps_{t}")
                    for e in range(NE):
                        nc.tensor.matmul(
                            vps[:, :],
                            vte[e][:, t * P:(t + 1) * P],
                            wv_t[e][:, :],
                            start=(e == 0), stop=(e == NE - 1),
                        )
                    for g in range(GROUPS_L):
                        nc.vector.tensor_copy(vh_aug[t][:, g, 0:D],
                                              vps[:, g * D:(g + 1) * D])
                    for g in range(GROUPS_L):
                        nc.vector.tensor_copy(vh_aug[t][:, g, D:D + 1],
                                              ones_col)

        # qte stream + wq chunk loads
        qte = []
        for e in range(NE):
            qt = p_in.tile([P, S], BF16, tag="in", name=f"qte_{e}")
            nc.sync.dma_start(out=qt, in_=qT[e * P:(e + 1) * P, :])
            qte.append(qt)

        def load_wq_chunk(o):
            tiles = []
            for e in range(NE):
                w = p_wq.tile([P, P], BF16, tag="wq", name=f"wq_{o}_{e}")
                nc.sync.dma_start(out=w,
                                  in_=wqT[e * P:(e + 1) * P, o * P:(o + 1) * P])
                tiles.append(w)
            return tiles

        # ---- phase 2: Q proj + attention slots + FC ------------------
        # PSUM: sc 2x2 banks + av 2 banks + qy 2 banks = 8 banks
        with tc.tile_pool(name="pssc", bufs=2, space="PSUM") as ps_sc, \
             tc.tile_pool(name="psav", bufs=1, space="PSUM") as ps_av, \
             tc.tile_pool(name="psqy", bufs=2, space="PSUM") as ps_qy:

            def q_proj_mms(wq_tiles, qps2, kcp):
                # 16e x 2t2 = 32 matmuls spread over kcp groups of 8
                for e in range(4 * kcp, 4 * kcp + 4):
                    for t2 in range(2):
                        nc.tensor.matmul(
                            qps2[t2][:, :],
                            wq_tiles[e][:, :],
                            qte[e][:, t2 * H2:(t2 + 1) * H2],
                            start=(e == 0), stop=(e == NE - 1),
                        )

            def drain_q(qps2, o):
                for t2 in range(2):
                    nc.vector.tensor_copy(
                        qh_t[o][:, t2 * H2:(t2 + 1) * H2], qps2[t2][:, :])

            def new_qps(o):
                return [ps_qy.tile([P, H2], F32, tag="pqy", name=f"qps_{o}_{t2}")
                        for t2 in range(2)]

            wq0 = load_wq_chunk(0)
            # Q chunk 0 up front (dense block, no fill needed yet)
            qps0 = new_qps(0)
            for kcp in range(4):
                q_proj_mms(wq0, qps0, kcp)
            drain_q(qps0, 0)
            wq_next = load_wq_chunk(1)

            def fc_chunk(eo, half):
                tsl = slice(half * H2, (half + 1) * H2)
                yps = ps_qy.tile([P, H2], F32, tag="pqy", name=f"yps_{half}_{eo}")
                for i in range(NT):
                    nc.tensor.matmul(
                        yps[:, :],
                        wfc_t[i][:, eo * P:(eo + 1) * P],
                        ot_t[i][:, tsl],
                        start=(i == 0), stop=(i == NT - 1),
                    )
                ysb = p_ysb.tile([P, H2], BF16, tag="ysb", name=f"ysb_{half}_{eo}")
                nc.vector.tensor_copy(ysb[:, :], yps[:, :])
                nc.sync.dma_start(out=y[eo * P:(eo + 1) * P, tsl], in_=ysb)

            for s in range(16):
                half, p = s // 8, s % 8
                g = p // 2
                tsl = slice(half * H2, (half + 1) * H2)
                if half == 0 and p < 7:
                    wq_cur = wq_next
                    if p < 6:
                        wq_next = load_wq_chunk(p + 2)
                    qps = new_qps(p + 1)
                av = ps_av.tile([P, 2, H2], F32, tag="psav", name=f"av_{s}")
                exps = []
                for kcp in range(4):
                    sc = [ps_sc.tile([P, 2, H2], F32, tag="pssc",
                                     name=f"sc_{s}_{kcp}_{i}") for i in range(2)]
                    for j in range(2):
                        kc = 2 * kcp + j
                        for i in range(2):
                            qb = i * D
                            nc.tensor.matmul(
                                sc[i][:, j, :],
                                kh_dup[g][qb:qb + D, kc * P:(kc + 1) * P],
                                qh_t[p][qb:qb + D, tsl],
                                start=True, stop=True,
                            )
                    ex = [p_exp.tile([P, 2, H2], BF16, tag="exp",
                                     name=f"exp_{s}_{kcp}_{i}") for i in range(2)]
                    for i in range(2):
                        nc.scalar.activation(
                            ex[i].rearrange("p a b -> p (a b)"),
                            sc[i].rearrange("p a b -> p (a b)"), AF.Exp)
                    exps.append(ex)
                    # fill work for this kcp
                    if half == 0 and p < 7:
                        q_proj_mms(wq_cur, qps, kcp)
                    elif half == 1 and kcp < 2:
                        fc_chunk(2 * p + kcp, 0)
                    # AV for the previous kcp (exp latency hidden)
                    if kcp > 0:
                        for j in range(2):
                            kc = 2 * (kcp - 1) + j
                            for i in range(2):
                                nc.tensor.matmul(
                                    av[0:D + 1, i, :],
                                    vh_aug[kc][:, g, :],
                                    exps[kcp - 1][i][:, j, :],
                                    start=(kc == 0), stop=False,
                                )
                for j in range(2):
                    kc = 6 + j
                    for i in range(2):
                        nc.tensor.matmul(
                            av[0:D + 1, i, :],
                            vh_aug[kc][:, g, :],
                            exps[3][i][:, j, :],
                            start=False, stop=(kc == NT - 1),
                        )
                if half == 0 and p < 7:
                    drain_q(qps, p + 1)
                # normalization: 1/den = Exp(-Ln(den)) on ACT (both heads in
                # one 1024-wide pass), broadcast, then DVE muls.
                rr = p_rr.tile([P, 2, H2], F32, tag="rr", name=f"rr_{s}")
                r2 = p_r2.tile([P, 2, H2], F32, tag="r2", name=f"r2_{s}")
                rb = p_rb.tile([P, 2, H2], F32, tag="rb", name=f"rb_{s}")
                nc.scalar.activation(rr[D:D + 1, :, :].rearrange("p a b -> p (a b)"),
                                     av[D:D + 1, :, :].rearrange("p a b -> p (a b)"),
                                     AF.Ln)
                nc.scalar.activation(r2[D:D + 1, :, :].rearrange("p a b -> p (a b)"),
                                     rr[D:D + 1, :, :].rearrange("p a b -> p (a b)"),
                                     AF.Exp, scale=-1.0)
                if _VARIANT["bcast_from"] == 0:
                    nc.gpsimd.dma_start(out=r2[0:1, :, :], in_=r2[D:D + 1, :, :])
                    nc.gpsimd.partition_broadcast(
                        rb[0:D, :, :].rearrange("p a b -> p (a b)"),
                        r2[0:1, :, :].rearrange("p a b -> p (a b)"))
                else:
                    nc.gpsimd.partition_broadcast(
                        rb[0:D, :, :].rearrange("p a b -> p (a b)"),
                        r2[D:D + 1, :, :].rearrange("p a b -> p (a b)"))
                nc.vector.tensor_mul(ot_t[p][0:D, tsl],
                                     av[0:D, 0, :], rb[0:D, 0, :])
                tmp = p_tmp.tile([P, H2], BF16, tag="tmp", name=f"tmp_{s}")
                nc.vector.tensor_mul(tmp[0:D, :],
                                     av[0:D, 1, :], rb[0:D, 1, :])
                nc.gpsimd.dma_start(out=ot_t[p][D:P, tsl], in_=tmp[0:D, :])

            # FC tail: token half B
            for eo in range(NE):
                fc_chunk(eo, 1)


def _get_nc():
    if "nc" not in _CACHE:
        _CACHE["nc"] = _build()
    return _CACHE["nc"]


def _in_maps(q, k, v, Wq, Wk, Wv, Wfc):
    bf = ml_dtypes.bfloat16
    qTb = [np.ascontiguousarray(q[b].T).astype(bf) for b in range(B)]
    kTb = [np.ascontiguousarray(k[b].T).astype(bf) for b in range(B)]
    vTb = [np.ascontiguousarray(v[b].T).astype(bf) for b in range(B)]
    wqTm = [np.ascontiguousarray((Wq[m * HO:(m + 1) * HO, :] / 8.0).T).astype(bf)
            for m in range(2)]
    wkTm = [np.ascontiguousarray(Wk[m * GO:(m + 1) * GO, :].T).astype(bf)
            for m in range(2)]
    wvTm = [np.ascontiguousarray(Wv[m * GO:(m + 1) * GO, :].T).astype(bf)
            for m in range(2)]
    wfcTm = [np.ascontiguousarray(Wfc[:, m * HO:(m + 1) * HO].T).astype(bf)
             for m in range(2)]
    maps = []
    for c in range(8):
        b, m = c // 2, c % 2
        maps.append({
            "qT": qTb[b], "kT": kTb[b], "vT": vTb[b],
            "wqT": wqTm[m], "wkT": wkTm[m], "wvT": wvTm[m],
            "wfcT": wfcTm[m],
        })
    return maps


def kernel(q, k, v, Wq, Wk, Wv, Wfc, bfc):
    q = np.asarray(q, np.float32)
    k = np.asarray(k, np.float32)
    v = np.asarray(v, np.float32)
    Wq = np.asarray(Wq, np.float32)
    Wk = np.asarray(Wk, np.float32)
    Wv = np.asarray(Wv, np.float32)
    Wfc = np.asarray(Wfc, np.float32)
    bfc = np.asarray(bfc, np.float32)

    nc = _get_nc()
    res = run_bass_kernel_spmd(nc, _in_maps(q, k, v, Wq, Wk, Wv, Wfc),
                               list(range(8)))
    out = np.empty((B, S, E), np.float32)
    for b in range(B):
        yt = (res.results[2 * b]["y"].astype(np.float32)
              + res.results[2 * b + 1]["y"].astype(np.float32))
        out[b] = yt.T + bfc
    return out


# revision 18
# speedup vs baseline: 1.4959x; 1.0929x over previous
"""GQA kernel for Trainium2, 8 NeuronCores (DP over batch x TP over heads).

Problem (hardcoded): B=4, S=1024, EMBED=2048, HEADS=32, GROUPS=8,
GROUP_HEADS=4, HEAD_DIM=64.

Core c handles batch b = c//2 and TP half m = c%2 (16 heads = 4 groups).
All matmul operands are bf16 (PSUM accumulation stays fp32); host converts.

Device pipeline (single dense PE stream to keep the HAM clock gate at 8/8):
  [K proj][V proj][Q proj chunk 0]
  [16 attention slots: slot s = head pair p=s%8, token half = s//8.
     scores for the pair are row-tiled (partitions 0:64 / 64:128) so the
     two heads' 64-contract matmuls run concurrently; one 2048-wide exp
     per kc-pair covers both heads (4 PSUM banks); AV accumulates
     [dims|ones] so softmax denominators fall out of the matmul;
     1/den = Exp(-Ln(den)) on ACT (activation tables pinned so exp+ln
     share one table set); gpsimd partition broadcast; DVE muls.
     Fill work keeps the PE busy under the ACT-bound exp stream:
     slots 0-6 run Q-proj chunk p+1, slots 8-15 run the output projection
     for token half A (2 out-chunks per slot).]
  [FC tail: output projection for token half B]
Output is yT [E, S] bf16 (stationary-wfc FC); host transposes and reduces.
"""

import numpy as np
import ml_dtypes

import concourse.bass as bass
import concourse.tile as tile
from concourse import bacc, mybir
from concourse.bass_utils import run_bass_kernel_spmd
from concourse.hw_specs import get_activation_tables

F32 = mybir.dt.float32
BF16 = mybir.dt.bfloat16
AF = mybir.ActivationFunctionType

B, S, E = 4, 1024, 2048
HEADS_L = 16          # heads per core
GROUPS_L = 4          # groups per core
D = 64                # head dim
P = 128
NE = E // P           # 16 e-chunks
NT = S // P           # 8 token chunks
HO = HEADS_L * D      # 1024 local head-dims
GO = GROUPS_L * D     # 256 local group-dims
H2 = S // 2           # 512 = token half

_CACHE = {}


def _pin_act_tables(arch):
    """Keep Exp/Ln only in natural_log_exp_and_others so the table-load
    pass picks the one set covering every activation this kernel uses
    (1 ACT_TABLE_LOAD instead of one per Ln<->Exp alternation). Mutates
    the functools.cache'd dict in place; set indices are unchanged."""
    tabs = get_activation_tables(arch)
    for name, fns in tabs.items():
        if name != "natural_log_exp_and_others":
            fns.discard(AF.Exp)
            fns.discard(AF.Ln)


def _build():
    nc = bacc.Bacc("TRN2")
    _pin_act_tables(nc.m.arch)
    qT = nc.declare_dram_parameter("qT", [E, S], BF16, isOutput=False)
    kT = nc.declare_dram_parameter("kT", [E, S], BF16, isOutput=False)
    vT = nc.declare_dram_parameter("vT", [E, S], BF16, isOutput=False)
    wqT = nc.declare_dram_parameter("wqT", [E, HO], BF16, isOutput=False)
    wkT = nc.declare_dram_parameter("wkT", [E, GO], BF16, isOutput=False)
    wvT = nc.declare_dram_parameter("wvT", [E, GO], BF16, isOutput=False)
    wfcT = nc.declare_dram_parameter("wfcT", [HO, E], BF16, isOutput=False)
    y = nc.declare_dram_parameter("y", [E, S], BF16, isOutput=True)

    with tile.TileContext(nc) as tc:
        _body(nc, tc, qT, kT, vT, wqT, wkT, wvT, wfcT, y)
    nc.finalize()
    return nc


def _body(nc, tc, qT, kT, vT, wqT, wkT, wvT, wfcT, y):
    from contextlib import ExitStack
    with ExitStack() as ctx:
        # ---- persistent SBUF pools -----------------------------------
        p_kh = ctx.enter_context(tc.tile_pool(name="kh", bufs=GROUPS_L))
        p_vh = ctx.enter_context(tc.tile_pool(name="vh", bufs=NT))
        p_qh = ctx.enter_context(tc.tile_pool(name="qh", bufs=NT))
        p_ot = ctx.enter_context(tc.tile_pool(name="ot", bufs=NT))
        p_wfc = ctx.enter_context(tc.tile_pool(name="wfc", bufs=NT))
        p_rr = ctx.enter_context(tc.tile_pool(name="rr", bufs=2))
        p_r2 = ctx.enter_context(tc.tile_pool(name="r2", bufs=2))
        p_rb = ctx.enter_context(tc.tile_pool(name="rb", bufs=2))
        p_tmp = ctx.enter_context(tc.tile_pool(name="tmp", bufs=2))
        p_ysb = ctx.enter_context(tc.tile_pool(name="ysb", bufs=4))
        p_exp = ctx.enter_context(tc.tile_pool(name="exp", bufs=6))
        # stream pool reused for kte -> vte -> qte (qte lives to slot 6)
        p_in = ctx.enter_context(tc.tile_pool(name="instream", bufs=4))
        p_wq = ctx.enter_context(tc.tile_pool(name="wq", bufs=2))

        kh_dup = [p_kh.tile([P, S], BF16, tag="kh", name=f"khdup_{g}")
                  for g in range(GROUPS_L)]
        vh_aug = [p_vh.tile([P, GROUPS_L, D + 1], BF16, tag="vh", name=f"vhaug_{t}")
                  for t in range(NT)]
        qh_t = [p_qh.tile([P, S], BF16, tag="qh", name=f"qh_{t}") for t in range(NT)]
        ot_t = [p_ot.tile([P, S], BF16, tag="ot", name=f"ot_{t}") for t in range(NT)]
        wfc_t = [p_wfc.tile([P, E], BF16, tag="wfc", name=f"wfc_{i}") for i in range(NT)]
        _CACHE["tiles"] = {"kh": kh_dup, "vh": vh_aug, "qh": qh_t, "ot": ot_t}

        ones_col = nc.const_aps.tensor(1.0, (P, 1), BF16)

        # e-major views of the transposed inputs/weights for batched DMA
        kT_r = kT.rearrange("(e p) t -> p e t", p=P)
        vT_r = vT.rearrange("(e p) t -> p e t", p=P)
        qT_r = qT.rearrange("(e p) t -> p e t", p=P)
        wkT_r = wkT.rearrange("(e p) c -> p e c", p=P)
        wvT_r = wvT.rearrange("(e p) c -> p e c", p=P)
        wqT_r = wqT.rearrange("(e p) c -> p e c", p=P)

        def load_in(src_r, nm):
            """4 tiles of [P, 4, S]; one DMA each."""
            tiles = []
            for q4 in range(4):
                t4 = p_in.tile([P, 4, S], BF16, tag="in", name=f"{nm}_{q4}")
                nc.sync.dma_start(out=t4, in_=src_r[:, 4 * q4:4 * q4 + 4, :])
                tiles.append(t4)
            return lambda e, sl: tiles[e // 4][:, e % 4, sl]

        # ---- phase 1: K/V projections --------------------------------
        with tc.tile_pool(name="wkv", bufs=1) as p_wkv:
            wk_all = p_wkv.tile([P, NE, GO], BF16, tag="wk", name="wk_all")
            nc.sync.dma_start(out=wk_all, in_=wkT_r)
            kte = load_in(kT_r, "kte")
            wv_all = p_wkv.tile([P, NE, GO], BF16, tag="wv", name="wv_all")
            nc.sync.dma_start(out=wv_all, in_=wvT_r)
            # wfc loads early; consumed from slot 8 on
            for i in range(NT):
                nc.sync.dma_start(out=wfc_t[i], in_=wfcT[i * P:(i + 1) * P, :])

            # K projection: khT [256,1024]; o2-outer so drains overlap
            with tc.tile_pool(name="pk", bufs=2, space="PSUM") as ps_k:
                for o2 in range(2):
                    khps = ps_k.tile([P, S], F32, tag="pk", name=f"khps_{o2}")
                    for e in range(NE):
                        for t2 in range(2):
                            nc.tensor.matmul(
                                khps[:, t2 * H2:(t2 + 1) * H2],
                                wk_all[:, e, o2 * P:(o2 + 1) * P],
                                kte(e, slice(t2 * H2, (t2 + 1) * H2)),
                                start=(e == 0), stop=(e == NE - 1),
                            )
                    nc.scalar.activation(kh_dup[2 * o2][0:D, :],
                                         khps[0:D, :], AF.Copy)
                    nc.scalar.activation(kh_dup[2 * o2 + 1][D:P, :],
                                         khps[D:P, :], AF.Copy)
                for g in range(GROUPS_L):
                    if g % 2 == 0:
                        nc.gpsimd.dma_start(out=kh_dup[g][D:P, :],
                                            in_=kh_dup[g][0:D, :])
                    else:
                        nc.gpsimd.dma_start(out=kh_dup[g][0:D, :],
                                            in_=kh_dup[g][D:P, :])

                # V projection: vh [tokens, dims]
                vte = load_in(vT_r, "vte")
                for t in range(NT):
                    vps = ps_k.tile([P, GO], F32, tag="pv", bufs=4,
                                    name=f"vps_{t}")
                    for e in range(NE):
                        nc.tensor.matmul(
                            vps[:, :],
                            vte(e, slice(t * P, (t + 1) * P)),
                            wv_all[:, e, :],
                            start=(e == 0), stop=(e == NE - 1),
                        )
                    for g in range(GROUPS_L):
                        nc.vector.tensor_copy(vh_aug[t][:, g, 0:D],
                                              vps[:, g * D:(g + 1) * D])
                    for g in range(GROUPS_L):
                        nc.vector.tensor_copy(vh_aug[t][:, g, D:D + 1],
                                              ones_col)

        qte = load_in(qT_r, "qte")

        def load_wq_chunk(o):
            w = p_wq.tile([P, NE, P], BF16, tag="wq", name=f"wq_{o}")
            nc.sync.dma_start(out=w, in_=wqT_r[:, :, o * P:(o + 1) * P])
            return w

        # ---- phase 2: Q proj + attention slots + FC ------------------
        # PSUM: sc 4 banks + av 2 banks + qy 2 banks = 8 banks
        with tc.tile_pool(name="pssc", bufs=1, space="PSUM") as ps_sc, \
             tc.tile_pool(name="psav", bufs=1, space="PSUM") as ps_av, \
             tc.tile_pool(name="psqy", bufs=2, space="PSUM") as ps_qy:

            def q_proj_mms(wq_tile, qps2, kcp):
                # 16e x 2t2 = 32 matmuls spread over kcp groups of 8
                for e in range(4 * kcp, 4 * kcp + 4):
                    for t2 in range(2):
                        nc.tensor.matmul(
                            qps2[t2][:, :],
                            wq_tile[:, e, :],
                            qte(e, slice(t2 * H2, (t2 + 1) * H2)),
                            start=(e == 0), stop=(e == NE - 1),
                        )

            def drain_q(qps2, o):
                for t2 in range(2):
                    nc.vector.tensor_copy(
                        qh_t[o][:, t2 * H2:(t2 + 1) * H2], qps2[t2][:, :])

            def new_qps(o):
                return [ps_qy.tile([P, H2], F32, tag="pqy", name=f"qps_{o}_{t2}")
                        for t2 in range(2)]

            wq0 = load_wq_chunk(0)
            # Q chunk 0 up front (dense block, no fill needed yet)
            qps0 = new_qps(0)
            for kcp in range(4):
                q_proj_mms(wq0, qps0, kcp)
            drain_q(qps0, 0)
            wq_next = load_wq_chunk(1)

            def fc_chunk(eo, half):
                tsl = slice(half * H2, (half + 1) * H2)
                yps = ps_qy.tile([P, H2], F32, tag="pqy", name=f"yps_{half}_{eo}")
                for i in range(NT):
                    nc.tensor.matmul(
                        yps[:, :],
                        wfc_t[i][:, eo * P:(eo + 1) * P],
                        ot_t[i][:, tsl],
                        start=(i == 0), stop=(i == NT - 1),
                    )
                ysb = p_ysb.tile([P, H2], BF16, tag="ysb", name=f"ysb_{half}_{eo}")
                nc.vector.tensor_copy(ysb[:, :], yps[:, :])
                nc.sync.dma_start(out=y[eo * P:(eo + 1) * P, tsl], in_=ysb)

            for s in range(16):
                half, p = s // 8, s % 8
                g = p // 2
                tsl = slice(half * H2, (half + 1) * H2)
                if half == 0 and p < 7:
                    wq_cur = wq_next
                    if p < 6:
                        wq_next = load_wq_chunk(p + 2)
                    qps = new_qps(p + 1)
                av = ps_av.tile([P, 2, H2], F32, tag="psav", name=f"av_{s}")
                exps = []
                for kcp in range(4):
                    sc = ps_sc.tile([P, 4, H2], F32, tag="pssc",
                                    name=f"sc_{s}_{kcp}")
                    for j in range(2):
                        kc = 2 * kcp + j
                        for i in range(2):
                            qb = i * D
                            nc.tensor.matmul(
                                sc[:, 2 * i + j, :],
                                kh_dup[g][qb:qb + D, kc * P:(kc + 1) * P],
                                qh_t[p][qb:qb + D, tsl],
                                start=True, stop=True,
                            )
                    ex = p_exp.tile([P, 4, H2], BF16, tag="exp",
                                    name=f"exp_{s}_{kcp}")
                    nc.scalar.activation(
                        ex.rearrange("p a b -> p (a b)"),
                        sc.rearrange("p a b -> p (a b)"), AF.Exp)
                    exps.append(ex)
                    # fill work for this kcp
                    if half == 0 and p < 7:
                        q_proj_mms(wq_cur, qps, kcp)
                    elif half == 1 and kcp < 2:
                        fc_chunk(2 * p + kcp, 0)
                    # AV for the previous kcp (exp latency hidden)
                    if kcp > 0:
                        for j in range(2):
                            kc = 2 * (kcp - 1) + j
                            for i in range(2):
                                nc.tensor.matmul(
                                    av[0:D + 1, i, :],
                                    vh_aug[kc][:, g, :],
                                    exps[kcp - 1][:, 2 * i + j, :],
                                    start=(kc == 0), stop=False,
                                )
                for j in range(2):
                    kc = 6 + j
                    for i in range(2):
                        nc.tensor.matmul(
                            av[0:D + 1, i, :],
                            vh_aug[kc][:, g, :],
                            exps[3][:, 2 * i + j, :],
                            start=False, stop=(kc == NT - 1),
                        )
                if half == 0 and p < 7:
                    drain_q(qps, p + 1)
                # normalization: 1/den = Exp(-Ln(den)) on ACT (both heads in
                # one 1024-wide pass), broadcast, then DVE muls.
                rr = p_rr.tile([P, 2, H2], F32, tag="rr", name=f"rr_{s}")
                r2 = p_r2.tile([P, 2, H2], F32, tag="r2", name=f"r2_{s}")
                rb = p_rb.tile([P, 2, H2], F32, tag="rb", name=f"rb_{s}")
                nc.scalar.activation(rr[D:D + 1, :, :].rearrange("p a b -> p (a b)"),
                                     av[D:D + 1, :, :].rearrange("p a b -> p (a b)"),
                                     AF.Ln)
                nc.scalar.activation(r2[D:D + 1, :, :].rearrange("p a b -> p (a b)"),
                                     rr[D:D + 1, :, :].rearrange("p a b -> p (a b)"),
                                     AF.Exp, scale=-1.0)
                nc.gpsimd.dma_start(out=r2[0:1, :, :], in_=r2[D:D + 1, :, :])
                nc.gpsimd.partition_broadcast(
                    rb[0:D, :, :].rearrange("p a b -> p (a b)"),
                    r2[0:1, :, :].rearrange("p a b -> p (a b)"))
                nc.vector.tensor_mul(ot_t[p][0:D, tsl],
                                     av[0:D, 0, :], rb[0:D, 0, :])
                tmp = p_tmp.tile([P, H2], BF16, tag="tmp", name=f"tmp_{s}")
                nc.vector.tensor_mul(tmp[0:D, :],
                                     av[0:D, 1, :], rb[0:D, 1, :])
                nc.gpsimd.dma_start(out=ot_t[p][D:P, tsl], in_=tmp[0:D, :])

            # FC tail: token half B
            for eo in range(NE):
                fc_chunk(eo, 1)


def _get_nc():
    if "nc" not in _CACHE:
        _CACHE["nc"] = _build()
    return _CACHE["nc"]


def _in_maps(q, k, v, Wq, Wk, Wv, Wfc):
    bf = ml_dtypes.bfloat16
    qTb = [np.ascontiguousarray(q[b].T).astype(bf) for b in range(B)]
    kTb = [np.ascontiguousarray(k[b].T).astype(bf) for b in range(B)]
    vTb = [np.ascontiguousarray(v[b].T).astype(bf) for b in range(B)]
    wqTm = [np.ascontiguousarray((Wq[m * HO:(m + 1) * HO, :] / 8.0).T).astype(bf)
            for m in range(2)]
    wkTm = [np.ascontiguousarray(Wk[m * GO:(m + 1) * GO, :].T).astype(bf)
            for m in range(2)]
    wvTm = [np.ascontiguousarray(Wv[m * GO:(m + 1) * GO, :].T).astype(bf)
            for m in range(2)]
    wfcTm = [np.ascontiguousarray(Wfc[:, m * HO:(m + 1) * HO].T).astype(bf)
             for m in range(2)]
    maps = []
    for c in range(8):
        b, m = c // 2, c % 2
        maps.append({
            "qT": qTb[b], "kT": kTb[b], "vT": vTb[b],
            "wqT": wqTm[m], "wkT": wkTm[m], "wvT": wvTm[m],
            "wfcT": wfcTm[m],
        })
    return maps


def kernel(q, k, v, Wq, Wk, Wv, Wfc, bfc):
    q = np.asarray(q, np.float32)
    k = np.asarray(k, np.float32)
    v = np.asarray(v, np.float32)
    Wq = np.asarray(Wq, np.float32)
    Wk = np.asarray(Wk, np.float32)
    Wv = np.asarray(Wv, np.float32)
    Wfc = np.asarray(Wfc, np.float32)
    bfc = np.asarray(bfc, np.float32)

    nc = _get_nc()
    res = run_bass_kernel_spmd(nc, _in_maps(q, k, v, Wq, Wk, Wv, Wfc),
                               list(range(8)))
    out = np.empty((B, S, E), np.float32)
    for b in range(B):
        yt = (res.results[2 * b]["y"].astype(np.float32)
              + res.results[2 * b + 1]["y"].astype(np.float32))
        out[b] = yt.T + bfc
    return out
